# revision 16
# baseline (speedup 1.0000x reference)
"""Deformable conv2d + residual add + ReLU on 8 Trainium2 NeuronCores.

Self-contained harness entry: kernel(**inputs) -> np.ndarray.
Sharding: data-parallel over batch N=8 (one image per core); weight/bias
replicated. Each core runs the same Bass/Tile program.

v2 pipeline (bf16 hot path, 5-engine balance):
  HOST: builds the bf16 sample table g4r[q] = [x|Dx|Dy|Dxy] (zero-padded,
    q-major rows), offset+base tensors in two device layouts, bf16 weights
    (lhsT), bf16 x2 in gather-order (m-order), and un-permutes the output.
  DEVICE per core:
    phase 1A (DVE): offsets -> clamped floor -> q indices -> i16 idx table
      (16-partition wrap via one strided SBUF DMA) for the SWDGE gather.
    phase 1B (Pool): same float pipeline in a [128,450] block layout ->
      per-(tap,block) bilinear fracs wx/wy as per-partition STT scalars.
    phase 3 per tap k: one dma_gather of 3136 rows (1KB bf16 each,
      alternating SWDGE queues); DVE STT: s = [x|Dy] + wx*[Dx|Dxy];
      Pool STT: colsP = s0 + wy*s1 (position-major); PE transposes via bf16
      identity into a 1-bank ping-pong PSUM stage; ACT copies to bf16
      channel-major cols; PE matmul accumulates into a 7-bank PSUM acc that
      was preloaded with x2 via an identity matmul.
    epilogue: ACT relu(acc + bias) -> f32 out, chunked DMA store.

Math: bilinear(x, py, px) = x[q] + wx*Dx[q] + wy*Dy[q] + wx*wy*Dxy[q] with
q = (floor(py)+PD)*WPAD + (floor(px)+PD) on the zero-padded grid; the zero
padding reproduces torchvision's out-of-bounds zeroing exactly, and clamping
floor() into the pad ring keeps fully-out-of-range samples at zero.
"""

import sys

for _p in ("/opt/trn_rl_repo",):
    if _p not in sys.path:
        sys.path.insert(0, _p)

import numpy as np
import ml_dtypes

import concourse.bacc as bacc
import concourse.mybir as mybir
import concourse.tile as tile
from concourse import bass_utils
from concourse.masks import make_identity

F32 = mybir.dt.float32
BF16 = mybir.dt.bfloat16
I32 = mybir.dt.int32
I16 = mybir.dt.int16
A = mybir.AluOpType
BF = ml_dtypes.bfloat16

# problem constants (nn_DeformConvAddReLU2d: N=8, C=Cout=128, 56x56, 3x3)
N, C, H, W = 8, 128, 56, 56
K = 9
PD = 2
HP, WP = H + 2 * PD, W + 2 * PD          # 60, 60
Q = HP * WP                               # 3600
NPOS = H * W                              # 3136
ELEM = 512                                # table row: [x|Dx|Dy|Dxy] x 128c
NIDX = 3200                               # gathered rows per tap (64 pad)
SLOTS = NIDX // 16                        # 200 wrapped idx slots per tap
NBLK = 25                                 # ceil(3136/128) landing blocks
CHUNK = 256
NCH = (NPOS + CHUNK - 1) // CHUNK         # 13 (12x256 + 64)
NMM = (NPOS + 511) // 512                 # 7 matmul chunks (6x512 + 64)

# m-order: sample slot m -> spatial position n  (m = (h1*28+s2)*16 + sp,
# n = h1*448 + sp*28 + s2).  This makes the idx-wrap DMA use 28-elem runs.
_m = np.arange(NPOS)
PERM = (_m // 16 // 28) * 448 + (_m % 16) * 28 + (_m // 16 % 28)
_mB = np.minimum(np.arange(NBLK * 128), NPOS - 1)
PERMB = PERM[_mB]


def host_bases():
    """base_y/base_x (+64 bias) per (tap, position): [2, K, NPOS] f32."""
    ki = np.arange(3).repeat(3)
    kj = np.tile(np.arange(3), 3)
    i = np.arange(H)
    j = np.arange(W)
    by = (i[None, :, None] - 1 + ki[:, None, None]).astype(np.float32)
    bx = (j[None, None, :] - 1 + kj[:, None, None]).astype(np.float32)
    by = np.broadcast_to(by, (K, H, W)).reshape(K, NPOS)
    bx = np.broadcast_to(bx, (K, H, W)).reshape(K, NPOS)
    return np.stack([by, bx]) + 64.0      # [2, K, NPOS]


def host_g4r(x_img):
    """x [C, NPOS] f32 -> bf16 table [Q, 512]: row q = [x|Dx|Dy|Dxy]."""
    xp = np.zeros((C, HP, WP), dtype=np.float32)
    xp[:, PD:PD + H, PD:PD + W] = x_img.reshape(C, H, W)
    xf = xp.reshape(C, Q)
    dx = np.zeros_like(xf)
    dy = np.zeros_like(xf)
    dxy = np.zeros_like(xf)
    dx[:, :Q - 1] = xf[:, 1:] - xf[:, :-1]
    dy[:, :Q - WP] = xf[:, WP:] - xf[:, :-WP]
    dxy[:, :Q - WP - 1] = dx[:, WP:Q - 1] - dx[:, :Q - WP - 1]
    planes = np.stack([xf, dx, dy, dxy])          # [4, C, Q]
    return np.ascontiguousarray(
        planes.transpose(2, 0, 1).reshape(Q, 4 * C)).astype(BF)


def make_core_inputs(x, offset, weight, bias, x2):
    """Full inputs -> list of 8 per-core input dicts (host batch sharding)."""
    baseP = host_bases()                           # [2, K, NPOS] (+64)
    wtb = np.ascontiguousarray(
        weight.reshape(C, C, K).transpose(1, 2, 0).reshape(C, K * C)
    ).astype(BF)
    basePA = np.zeros((128, 448), dtype=np.float32)
    basePA[0:63] = baseP[0].reshape(63, 448)
    basePA[64:127] = baseP[1].reshape(63, 448)
    # [128, 450] block layout: col a*225 + k*25 + b, partition p = sample%128
    basePB = np.ascontiguousarray(
        baseP[:, :, PERMB].reshape(2, K, NBLK, 128).transpose(3, 0, 1, 2)
        .reshape(128, 2 * K * NBLK), dtype=np.float32)
    cores = []
    for i in range(N):
        off3 = np.asarray(offset[i], dtype=np.float32).reshape(K, 2, NPOS)
        offA = np.zeros((128, 448), dtype=np.float32)
        offA[0:63] = off3[:, 0].reshape(63, 448)
        offA[64:127] = off3[:, 1].reshape(63, 448)
        offB = np.ascontiguousarray(
            off3.transpose(1, 0, 2)[:, :, PERMB].reshape(2, K, NBLK, 128)
            .transpose(3, 0, 1, 2).reshape(128, 2 * K * NBLK),
            dtype=np.float32)
        x2m = np.ascontiguousarray(
            np.asarray(x2[i], dtype=np.float32).reshape(C, NPOS)[:, PERM]
        ).astype(BF)
        cores.append({
            "g4r": host_g4r(np.asarray(x[i], dtype=np.float32).reshape(C, NPOS)),
            "offA": offA, "basePA": basePA,
            "offB": offB, "basePB": basePB,
            "x2m": x2m, "wt": wtb,
            "bias": np.ascontiguousarray(bias.reshape(C, 1), dtype=np.float32),
        })
    return cores


def build_kernel(tc, outs, ins):
    nc = tc.nc
    out_d = outs                                   # [128, NPOS] f32
    g4r_d, offA_d, basePA_d, offB_d, basePB_d, x2_d, wt_d, bias_d = ins

    with tc.tile_pool(name="persist", bufs=1) as pers, \
         tc.tile_pool(name="dram", bufs=1, space="DRAM") as dp:
        qfi_d = dp.tile([63, 448], I16)
        idnf = pers.tile([128, 128], F32)
        make_identity(nc, idnf[:])
        idn = pers.tile([128, 128], BF16)
        nc.vector.tensor_copy(out=idn[:], in_=idnf[:])
        w_sb = pers.tile([128, K * 128], BF16)     # lhsT per tap: [c, o]
        bias_sb = pers.tile([128, 1], F32)
        x2_sb = pers.tile([128, NPOS], BF16)       # m-order residual
        wsc = pers.tile([128, 2 * K * NBLK], F32)  # fracs: wy at k*25+b, wx at 225+
        idxw = pers.tile([128, K * SLOTS], I16)    # wrapped gather indices

        nc.sync.dma_start(out=w_sb[:], in_=wt_d[:])
        nc.sync.dma_start(out=bias_sb[:], in_=bias_d[:])
        nc.sync.dma_start(out=x2_sb[:], in_=x2_d[:])

        # ---------------- phase 1A: gather indices (DVE) ----------------
        with tc.tile_pool(name="p1", bufs=1) as sp:
            offA = sp.tile([128, 448], F32)
            nc.sync.dma_start(out=offA[:], in_=offA_d[:])
            basePA = sp.tile([128, 448], F32)
            nc.sync.dma_start(out=basePA[:], in_=basePA_d[:])
            pA = sp.tile([128, 448], F32)
            nc.vector.tensor_tensor(out=pA[:], in0=offA[:], in1=basePA[:],
                                    op=A.add)
            tcl = sp.tile([128, 448], F32)
            nc.vector.tensor_scalar(out=tcl[:], in0=pA[:], scalar1=62.0,
                                    scalar2=120.0, op0=A.max, op1=A.min)
            ri = sp.tile([128, 448], I32)
            nc.vector.tensor_copy(out=ri[:], in_=tcl[:])
            rf = sp.tile([128, 448], F32)
            nc.vector.tensor_copy(out=rf[:], in_=ri[:])
            gtt = sp.tile([128, 448], F32)
            nc.vector.tensor_tensor(out=gtt[:], in0=rf[:], in1=tcl[:],
                                    op=A.is_gt)
            fl = sp.tile([128, 448], F32)
            nc.vector.tensor_tensor(out=fl[:], in0=rf[:], in1=gtt[:],
                                    op=A.subtract)
            flx = sp.tile([63, 448], F32)
            nc.sync.dma_start(out=flx[:], in_=fl[64:127, :])
            qfA = sp.tile([63, 448], F32)
            nc.vector.tensor_scalar(out=qfA[:], in0=fl[0:63, :], scalar1=60.0,
                                    scalar2=-3782.0, op0=A.mult, op1=A.add)
            nc.vector.tensor_tensor(out=qfA[:], in0=qfA[:], in1=flx[:],
                                    op=A.add)
            qfi = sp.tile([63, 448], I16)
            nc.vector.tensor_copy(out=qfi[:], in_=qfA[:])
            # 16-partition wrap via DRAM: [(k h1), (sp s2)] -> [sp, (k h1 s2)]
            nc.sync.dma_start(out=qfi_d[:], in_=qfi[:])
            # pad slots (196..199 per tap) gather row 0 (all-zeros pad corner)
            nc.vector.memset(idxw[0:16, :], 0)
            for kk in range(K):
                nc.sync.dma_start(
                    out=idxw[0:16, kk * SLOTS:kk * SLOTS + 196].rearrange(
                        "sp (h1 s2) -> sp h1 s2", h1=7),
                    in_=qfi_d[kk * 7:(kk + 1) * 7, :].rearrange(
                        "h1 (sp s2) -> sp h1 s2", sp=16))
            nc.sync.dma_start(out=idxw[16:32, :], in_=idxw[0:16, :])
            nc.sync.dma_start(out=idxw[32:64, :], in_=idxw[0:32, :])
            nc.sync.dma_start(out=idxw[64:128, :], in_=idxw[0:64, :])

            # ------------- phase 1B: bilinear fracs (Pool) -------------
            offB = sp.tile([128, 450], F32)
            nc.sync.dma_start(out=offB[:], in_=offB_d[:])
            basePB = sp.tile([128, 450], F32)
            nc.sync.dma_start(out=basePB[:], in_=basePB_d[:])
            pB = sp.tile([128, 450], F32)
            nc.vector.tensor_tensor(out=pB[:], in0=offB[:], in1=basePB[:],
                                    op=A.add)
            tclB = sp.tile([128, 450], F32)
            nc.vector.tensor_scalar(out=tclB[:], in0=pB[:], scalar1=62.0,
                                    scalar2=120.0, op0=A.max, op1=A.min)
            riB = sp.tile([128, 450], I32)
            nc.vector.tensor_copy(out=riB[:], in_=tclB[:])
            rfB = sp.tile([128, 450], F32)
            nc.vector.tensor_copy(out=rfB[:], in_=riB[:])
            gttB = sp.tile([128, 450], F32)
            nc.vector.tensor_tensor(out=gttB[:], in0=rfB[:], in1=tclB[:],
                                    op=A.is_gt)
            flB = sp.tile([128, 450], F32)
            nc.vector.tensor_tensor(out=flB[:], in0=rfB[:], in1=gttB[:],
                                    op=A.subtract)
            nc.vector.tensor_tensor(out=wsc[:], in0=pB[:], in1=flB[:],
                                    op=A.subtract)

        # ---------------- phase 3: gather / combine / matmul ----------------
        with tc.tile_pool(name="gk", bufs=2) as gp, \
             tc.tile_pool(name="sp3", bufs=2) as spp, \
             tc.tile_pool(name="cp", bufs=2) as cpp, \
             tc.tile_pool(name="cols", bufs=2) as csp, \
             tc.tile_pool(name="ep", bufs=1) as epp, \
             tc.tile_pool(name="accp", bufs=1, space="PSUM") as accp, \
             tc.tile_pool(name="stg", bufs=1, space="PSUM") as stgp:
            acc = accp.tile([128, NPOS], F32)      # 7 banks
            stage = stgp.tile([128, 512], BF16)    # half bank, ping-pong halves

            # preload acc with x2 (identity matmul; 512-wide bank-aligned
            # accumulation groups, started here, closed by tap K-1 matmuls)
            for j in range(NMM):
                lo = 512 * j
                hi = min(lo + 512, NPOS)
                nc.tensor.matmul(acc[:, lo:hi], lhsT=idn[:],
                                 rhs=x2_sb[:, lo:hi], start=True, stop=False)

            gk_tiles = {}

            halves = [(0, 8), (8, 8), (16, 8), (24, 1)]

            def emit_gather(k):
                gk = gp.tile([128, NBLK, ELEM], BF16, tag="gk", name=f"gk{k}")
                for hb, nb in halves:
                    nc.gpsimd.dma_gather(
                        gk[:, hb:hb + nb, :], g4r_d[:],
                        idxw[:, k * SLOTS + hb * 8:k * SLOTS + (hb + nb) * 8],
                        num_idxs=nb * 128, num_idxs_reg=nb * 128,
                        elem_size=ELEM, queue_num=0)
                gk_tiles[k] = gk

            emit_gather(0)
            emit_gather(1)
            half = 0
            for k in range(K):
                gk = gk_tiles.pop(k)
                # planes within a row: [x|Dx|Dy|Dxy] -> (two=2, pair=2, c=128)
                gkv = gk[:].rearrange("p nb (two pair c) -> p nb two pair c",
                                      two=2, pair=2)
                s = spp.tile([128, NBLK, 2, 128], BF16, tag="s", name=f"s{k}")
                colsP = cpp.tile([128, NBLK, 128], BF16, tag="cP",
                                 name=f"cP{k}")
                for b in range(NBLK):
                    # s = [x|Dy] + wx * [Dx|Dxy]
                    nc.vector.scalar_tensor_tensor(
                        out=s[:, b, :, :], in0=gkv[:, b, :, 1, :],
                        scalar=wsc[:, 225 + k * 25 + b:226 + k * 25 + b],
                        in1=gkv[:, b, :, 0, :], op0=A.mult, op1=A.add)
                if k + 1 < K:
                    emit_gather(k + 1)
                for b in range(NBLK):
                    # colsP = s0 + wy * s1
                    nc.vector.scalar_tensor_tensor(
                        out=colsP[:, b, :], in0=s[:, b, 1, :],
                        scalar=wsc[:, k * 25 + b:1 + k * 25 + b],
                        in1=s[:, b, 0, :], op0=A.mult, op1=A.add)
                cols = csp.tile([128, NPOS], BF16, tag="cols", name=f"co{k}")
                for j in range(NCH):
                    lo = CHUNK * j
                    hi = min(lo + CHUNK, NPOS)
                    hoff = half * 256
                    half ^= 1
                    for jj in range((hi - lo + 127) // 128):
                        nc.tensor.transpose(
                            out=stage[:, hoff + 128 * jj:hoff + 128 * (jj + 1)],
                            in_=colsP[:, 2 * j + jj, :], identity=idn[:])
                    nc.scalar.copy(out=cols[:, lo:hi],
                                   in_=stage[:, hoff:hoff + (hi - lo)])
                    if j % 2 == 1 or hi == NPOS:
                        mlo = 512 * (j // 2)
                        nc.tensor.matmul(acc[:, mlo:hi],
                                         lhsT=w_sb[:, k * 128:(k + 1) * 128],
                                         rhs=cols[:, mlo:hi],
                                         start=False, stop=(k == K - 1))

            # ---------------- epilogue ----------------
            outsb = epp.tile([128, NPOS], F32)
            for j in range(NCH):
                lo = CHUNK * j
                hi = min(lo + CHUNK, NPOS)
                nc.scalar.activation(outsb[:, lo:hi], acc[:, lo:hi],
                                     mybir.ActivationFunctionType.Relu,
                                     bias=bias_sb[:], scale=1.0)
            qtr = NPOS // 4
            cuts = [0, qtr, 2 * qtr, 3 * qtr, NPOS]
            for j in range(4):
                nc.sync.dma_start(out=out_d[:, cuts[j]:cuts[j + 1]],
                                  in_=outsb[:, cuts[j]:cuts[j + 1]])


IN_SPECS = [("g4r", (Q, ELEM), BF16), ("offA", (128, 448), F32),
            ("basePA", (128, 448), F32), ("offB", (128, 450), F32),
            ("basePB", (128, 450), F32), ("x2m", (C, NPOS), BF16),
            ("wt", (C, K * 128), BF16), ("bias", (C, 1), F32)]

_CACHED_NC = None


def _build_nc():
    global _CACHED_NC
    if _CACHED_NC is not None:
        return _CACHED_NC
    nc = bacc.Bacc("TRN2", target_bir_lowering=False, debug=False,
                   num_devices=N)
    ins = [nc.dram_tensor(nm, list(sh), dt, kind="ExternalInput").ap()
           for nm, sh, dt in IN_SPECS]
    out = nc.dram_tensor("out", [C, NPOS], F32, kind="ExternalOutput").ap()
    with tile.TileContext(nc, trace_sim=False) as tc:
        build_kernel(tc, out, ins)
    nc.compile()
    _CACHED_NC = nc
    return nc


def run_cores(inputs, trace=False):
    """Run the SPMD kernel; returns (out [N,C,H,W] f32, exec_time_ns or None)."""
    nc = _build_nc()
    in_maps = make_core_inputs(inputs["x"], inputs["offset"], inputs["weight"],
                               inputs["bias"], inputs["x2"])
    res = bass_utils.run_bass_kernel_spmd(nc, in_maps, core_ids=list(range(N)),
                                          trace=trace)
    out = np.empty((N, C, NPOS), dtype=np.float32)
    for i in range(N):
        out[i][:, PERM] = res.results[i]["out"]
    return out.reshape(N, C, H, W), res.exec_time_ns


def kernel(x, offset, weight, bias, x2):
    x = np.asarray(x, dtype=np.float32)
    offset = np.asarray(offset, dtype=np.float32)
    weight = np.asarray(weight, dtype=np.float32)
    bias = np.asarray(bias, dtype=np.float32)
    x2 = np.asarray(x2, dtype=np.float32)
    out, _ = run_cores({"x": x, "offset": offset, "weight": weight,
                        "bias": bias, "x2": x2}, trace=False)
    return out
